# revision 1
# baseline (speedup 1.0000x reference)
"""Batched int8 GEMM (s8t x s8n -> s32t) on 8 TRN2 NeuronCores.

out[b, m, n] = sum_k a[b, m, k] * b[b, n, k]   (int32 accumulation)
a: [32, 1024, 1024] int8, b: [32, 1024, 1024] int8 -> out: [32, 1024, 1024] int32

Strategy:
  - Pure batch parallelism: 4 batches per core across 8 cores.
  - Both operands have K innermost, but the PE needs K on partitions.
    DMA-transpose works on 2-byte elements only, so we view the int8
    inputs as uint16 (pairs of adjacent K values) and DMA-transpose
    per-batch K-blocks of 256 K-values ([1024, 128] uint16 ->
    [128, 1024]), each partition holding an even/odd K pair interleaved
    along the free dim. Per-batch (rather than batch-pair) transposes
    keep each transpose's consumers within one batch so its DMA
    semaphore lane recycles quickly (8 HWDGE lanes rotate over all
    HWDGE DMAs; a lane is not reusable until the prior user's consumers
    have executed).
  - DVE deinterleaves (stride-2 int8 reads) and converts int8 -> bf16.
    int8 is exactly representable in bf16; products <= 2^14 and sums
    <= 2^24 are exact in fp32 PSUM accumulation, so the GEMM is
    bit-exact (native int8 matmul is rejected by walrus's BIR verifier,
    so bf16 is the fastest exact path).
  - PE: bf16 matmuls, K=128 per instruction, 8-step accumulation into
    [128, 512] fp32 PSUM banks (8 banks in flight). 16 dummy matmuls up
    front warm the HAM clock gate, and a few filler matmuls pad batch
    0's first mt block (which is rate-limited by the deint stream) so
    the PE never idles long enough to re-throttle.
  - ACT copies PSUM fp32 -> SBUF int32 (exact: values are integers) and
    issues one 4MiB output store per batch (HWDGE; sparse stores never
    collide with transpose semaphore lanes); the last batch stores
    per-mt so the kernel tail only waits on the final 512KB. SYNC
    issues only transposes, DVE only deints. The strict engine
    separation avoids FIFO head-of-line blocking between the deint
    stream, the PSUM-freeing stream, and the DMA streams.
"""

import numpy as np

import concourse.bass as bass
import concourse.mybir as mybir
import concourse.tile as tile
from concourse import bacc
from concourse.bass_utils import run_bass_kernel_spmd
from concourse.tile_rust import add_dep_helper

B, M, N, K = 32, 1024, 1024, 1024
N_CORES = 8
BPC = B // N_CORES  # batches per core
KB = K // 256  # k-blocks of 256 K-values (128 uint16 partitions)
N_TILE = 512
M_TILE = 128

_nc_cache = None


def build_nc():
    nc = bacc.Bacc("TRN2")

    # int8 inputs viewed as uint16 so the xbar DMA-transpose (2-byte
    # granularity) can be used straight out of HBM.
    a_in = nc.dram_tensor("a", [BPC, M, K // 2], mybir.dt.uint16, kind="ExternalInput")
    b_in = nc.dram_tensor("b", [BPC, N, K // 2], mybir.dt.uint16, kind="ExternalInput")
    out = nc.dram_tensor("out", [BPC, M, N], mybir.dt.int32, kind="ExternalOutput")

    with tile.TileContext(nc) as tc:
        with (
            tc.tile_pool(name="stage", bufs=2) as stage_pool,
            tc.tile_pool(name="conv", bufs=2) as conv_pool,
            tc.tile_pool(name="psum", bufs=8, space="PSUM") as psum_pool,
            tc.tile_pool(name="outbuf", bufs=2) as out_pool,
            tc.tile_pool(name="warm", bufs=1) as warm_pool,
        ):
            # PE warmup: dummy matmuls with NO deps at all (uninitialized
            # SBUF reads are fine; the PSUM result is discarded), so the
            # HAM clock gate reaches K=8/8 before the real MM stream starts.
            wsrc = warm_pool.tile([128, N_TILE], mybir.dt.bfloat16, name="wsrc")
            nc.vector.memset(wsrc[:, :8], 0.0)
            wps = psum_pool.tile([128, N_TILE], mybir.dt.float32, name="wps", tag="ps")
            for _ in range(16):
                nc.tensor.matmul(wps[:], wsrc[:, :128], wsrc[:], start=True, stop=True)

            # Batch stores are deferred into the middle of the NEXT batch's
            # copy stream: the store's semaphore-lane WAR event-sem then
            # resolves during ACT idle time instead of blocking the
            # PSUM-freeing copies the PE is waiting on.
            pending_store = None
            for bi in range(BPC):
                # ---- per-batch DMA-transpose staging: each transpose's
                # consumers (2 deints) execute within this batch's prologue,
                # so its semaphore lane recycles quickly. ----
                a_st = []
                b_st = []
                for kb in range(KB):
                    at = stage_pool.tile(
                        [128, M], mybir.dt.uint16, name=f"at_{bi}_{kb}", tag=f"at{kb}"
                    )
                    nc.sync.dma_start_transpose(at[:], a_in[bi, :, kb * 128 : (kb + 1) * 128])
                    a_st.append(at.bitcast(mybir.dt.int8))
                    bt = stage_pool.tile(
                        [128, N], mybir.dt.uint16, name=f"bt_{bi}_{kb}", tag=f"bt{kb}"
                    )
                    nc.sync.dma_start_transpose(bt[:], b_in[bi, :, kb * 128 : (kb + 1) * 128])
                    b_st.append(bt.bitcast(mybir.dt.int8))

                # ---- deinterleave + int8 -> bf16 (DVE) ----
                a_bf = []  # 8 bf16 tiles [128, M]; k-tile = kb*2+parity
                b_bf = []
                for kb in range(KB):
                    for par in range(2):
                        abf = conv_pool.tile(
                            [128, M],
                            mybir.dt.bfloat16,
                            name=f"abf_{bi}_{kb}_{par}",
                            tag=f"abf{kb}{par}",
                        )
                        nc.vector.tensor_copy(abf[:], a_st[kb][:, par::2])
                        a_bf.append(abf)
                        bbf = conv_pool.tile(
                            [128, N],
                            mybir.dt.bfloat16,
                            name=f"bbf_{bi}_{kb}_{par}",
                            tag=f"bbf{kb}{par}",
                        )
                        nc.vector.tensor_copy(bbf[:], b_st[kb][:, par::2])
                        b_bf.append(bbf)

                # ---- GEMM, accumulating in PSUM over kt. All 8 mt blocks
                # copy into one big staging tile; a single 4MiB store per
                # batch keeps HWDGE store traffic too sparse to collide with
                # transpose semaphore lanes. ----
                n_kt = 2 * KB
                n_mt = M // M_TILE
                ot = out_pool.tile(
                    [128, n_mt, N], mybir.dt.int32, name=f"ot_{bi}", tag="ot"
                )
                if bi == 0:
                    # Batch 0 is rate-limited by the serial-xbar transpose +
                    # deint stream: iterate kt-outer over groups of 4 mt
                    # blocks (8 PSUM banks) so each arriving k-tile feeds
                    # 1.73us of real PE work and the ramp is gapless.
                    for g in range(n_mt // 4):
                        mts = range(4 * g, 4 * g + 4)
                        ps = {
                            (mt, nt): psum_pool.tile(
                                [128, N_TILE],
                                mybir.dt.float32,
                                name=f"ps_{bi}_{mt}_{nt}",
                                tag="ps",
                            )
                            for mt in mts
                            for nt in range(N // N_TILE)
                        }
                        for kt in range(n_kt):
                            for mt in mts:
                                lhsT = a_bf[kt][:, mt * M_TILE : (mt + 1) * M_TILE]
                                for nt in range(N // N_TILE):
                                    nc.tensor.matmul(
                                        ps[(mt, nt)][:],
                                        lhsT,
                                        b_bf[kt][:, nt * N_TILE : (nt + 1) * N_TILE],
                                        start=(kt == 0),
                                        stop=(kt == n_kt - 1),
                                    )
                        for mt in mts:
                            for nt in range(N // N_TILE):
                                nc.scalar.copy(
                                    ot[:, mt, nt * N_TILE : (nt + 1) * N_TILE],
                                    ps[(mt, nt)][:],
                                )
                else:
                    # Steady-state batches: mt-outer so the PSUM-freeing
                    # copies spread evenly instead of bunching.
                    for mt in range(n_mt):
                        ps = [
                            psum_pool.tile(
                                [128, N_TILE],
                                mybir.dt.float32,
                                name=f"ps_{bi}_{mt}_{nt}",
                                tag="ps",
                            )
                            for nt in range(N // N_TILE)
                        ]
                        for kt in range(n_kt):
                            lhsT = a_bf[kt][:, mt * M_TILE : (mt + 1) * M_TILE]
                            for nt in range(N // N_TILE):
                                nc.tensor.matmul(
                                    ps[nt][:],
                                    lhsT,
                                    b_bf[kt][:, nt * N_TILE : (nt + 1) * N_TILE],
                                    start=(kt == 0),
                                    stop=(kt == n_kt - 1),
                                )
                        # fp32 -> int32 PSUM-freeing copies on ACT. For the
                        # very last mt block, the second copy goes on the
                        # (by then idle) DVE so the two copies run in
                        # parallel and the final store starts ~0.7us sooner.
                        last_copy = None
                        if bi == BPC - 1 and mt == n_mt - 1:
                            nc.scalar.copy(ot[:, mt, :N_TILE], ps[0][:])
                            nc.vector.tensor_copy(ot[:, mt, N_TILE:], ps[1][:])
                        else:
                            for nt in range(N // N_TILE):
                                last_copy = nc.scalar.copy(
                                    ot[:, mt, nt * N_TILE : (nt + 1) * N_TILE], ps[nt][:]
                                )
                        if mt == 3 and pending_store is not None:
                            pbi, pot = pending_store
                            st = nc.scalar.dma_start(
                                out[pbi].rearrange("(t p) n -> p t n", p=128), pot[:]
                            )
                            # Ordering-only edge: keep the store (and its
                            # semaphore-lane WAR wait) behind this batch's
                            # mt0-3 copies in the ACT FIFO.
                            add_dep_helper(
                                st.ins,
                                last_copy.ins,
                                False,
                                "defer batch store past next batch's early copies",
                            )
                            pending_store = None
                        if bi == BPC - 1:
                            # Last batch: store per mt so the tail only waits
                            # on the final 512KB.
                            nc.scalar.dma_start(
                                out[bi, mt * M_TILE : (mt + 1) * M_TILE, :], ot[:, mt, :]
                            )
                if bi < BPC - 1:
                    # One 4MiB store for the whole batch: HBM row
                    # (mt*128 + p) pairs with SBUF [p, mt, :]. Deferred
                    # (emitted mid-next-batch, see above).
                    pending_store = (bi, ot)
    nc.compile()
    return nc


def _get_nc():
    global _nc_cache
    if _nc_cache is None:
        _nc_cache = build_nc()
    return _nc_cache


def run(a: np.ndarray, b: np.ndarray, trace: bool = False):
    """Run on 8 cores. a/b: [32, 1024, 1024] int8. Returns (out, BassKernelResults)."""
    a = np.ascontiguousarray(a)
    b = np.ascontiguousarray(b)
    a16 = a.view(np.uint16).reshape(B, M, K // 2)
    b16 = b.view(np.uint16).reshape(B, N, K // 2)
    in_maps = [
        {
            "a": a16[c * BPC : (c + 1) * BPC],
            "b": b16[c * BPC : (c + 1) * BPC],
        }
        for c in range(N_CORES)
    ]
    res = run_bass_kernel_spmd(_get_nc(), in_maps, list(range(N_CORES)), trace=trace)
    out = np.concatenate([res.results[c]["out"] for c in range(N_CORES)], axis=0)
    return out, res


def kernel(a: np.ndarray, b: np.ndarray) -> np.ndarray:
    out, _ = run(np.asarray(a), np.asarray(b))
    return out

